# revision 23
# baseline (speedup 1.0000x reference)
"""Trainium2 Bass kernel for nn_CoreBlock (circulant attention + 2-layer FFN).

Contract: kernel(**inputs) takes FULL unsharded inputs (as produced by
setup_inputs) and returns the FULL [16, 1024, 768] f32 output.

Strategy: pure data-parallel over batch — 8 NeuronCores x 2 batches each.
All weights replicated. Per core:
  phase A: LayerNorm(x) -> u (normalize on the ACT engine with per-token
           scale/bias), PE-transpose u, v = u_dt.T @ Wv' per token chunk,
           gathered into a resident V tensor [128, H, B, NJ, HS] in SBUF.
  phase B: per head h and batch b: circulant matmul via the 8-tile Toeplitz
           bank; each diagonal m is 2 wide matmuls (moving free up to 512)
           instead of 8 narrow ones, so the PE streams at full rate instead
           of being weight-load-bound. Residual-added into X (X -> x1).
  phase C: 2x [Dense -> LayerNorm -> swish]; layer 2 runs in two halves so
           the log_cosh tail of half A overlaps half B's matmuls. Tail uses
           log_cosh(w) = w + softplus(-2w) - log2 (one table), with the
           final adds split between DVE and the idle GpSimd engine.

Matmul operands are bf16 (full-rate PE, fp32 PSUM accumulation); stats and
elementwise math are fp32. All Sqrt ops are batched so the scalar engine's
activation-table reloads stay rare.
"""

import math
import numpy as np
import ml_dtypes

import concourse.bass as bass
import concourse.tile as tile
from concourse import bacc, mybir
from concourse.bass_utils import run_bass_kernel_spmd

BF16 = ml_dtypes.bfloat16

B, N, D = 16, 1024, 768
H, HS, L = 12, 64, 2
EPS = 1e-6
NCORES = 8
BPC = B // NCORES          # batches per core
NJ = N // 128              # token chunks per batch (8)
NT = BPC * NJ              # token chunks per core (16)
DC = D // 128              # feature chunks (6)

F32 = mybir.dt.float32
BF = mybir.dt.bfloat16
Alu = mybir.AluOpType
Act = mybir.ActivationFunctionType

TRACE = False              # test harness sets this for profiling runs
TRACE_KW = {}

_cache = {}


def _build(cv_nonzero, bf_nonzero, lnf_uniform):
    """Construct the per-core Bass program. lnf_uniform: per-layer (cs, cb)
    if lnf scale/bias are uniform scalars, else None entries."""
    nc = bacc.Bacc("TRN2", target_bir_lowering=False, debug=False)

    xs = nc.dram_tensor("xs", (BPC, N, D), F32, kind="ExternalInput").ap()
    wv = nc.dram_tensor("wv", (D, D), BF, kind="ExternalInput").ap()
    wf = nc.dram_tensor("wf", (L, D, D), BF, kind="ExternalInput").ap()
    tb_d = nc.dram_tensor("tbank", (H, 128, NJ * 128), BF, kind="ExternalInput").ap()
    id32 = nc.dram_tensor("id32", (128, 128), F32, kind="ExternalInput").ap()
    idbf = nc.dram_tensor("idbf", (128, 128), BF, kind="ExternalInput").ap()
    cv_d = nc.dram_tensor("cv", (D,), F32, kind="ExternalInput").ap()
    bf_d = nc.dram_tensor("bfb", (L, D), F32, kind="ExternalInput").ap()
    lnfs_d = nc.dram_tensor("lnfs", (L, D), F32, kind="ExternalInput").ap()
    lnfb_d = nc.dram_tensor("lnfb", (L, D), F32, kind="ExternalInput").ap()
    out_d = nc.dram_tensor("out", (BPC, N, D), F32, kind="ExternalOutput").ap()

    with tile.TileContext(nc) as tc:
        _emit(nc, tc, xs, wv, wf, tb_d, id32, idbf, cv_d, bf_d, lnfs_d, lnfb_d,
              out_d, cv_nonzero, bf_nonzero, lnf_uniform)
    nc.compile()
    return nc


def _emit(nc, tc, xs, wv, wf, tb_d, id32, idbf, cv_d, bf_d, lnfs_d, lnfb_d,
          out_d, cv_nonzero, bf_nonzero, lnf_uniform):
    from contextlib import ExitStack
    ctx = ExitStack()
    with ctx:
        consts = ctx.enter_context(tc.tile_pool(name="consts", bufs=1))
        xpool = ctx.enter_context(tc.tile_pool(name="xpool", bufs=1))
        vpool = ctx.enter_context(tc.tile_pool(name="vpool", bufs=1))
        acts = ctx.enter_context(tc.tile_pool(name="acts", bufs=18))
        upool = ctx.enter_context(tc.tile_pool(name="upool", bufs=3))
        dtp = ctx.enter_context(tc.tile_pool(name="dtp", bufs=3))
        stat = ctx.enter_context(tc.tile_pool(name="stat", bufs=4))
        statp = ctx.enter_context(tc.tile_pool(name="statp", bufs=2))
        wkp = ctx.enter_context(tc.tile_pool(name="wkp", bufs=3))
        spp = ctx.enter_context(tc.tile_pool(name="spp", bufs=2))
        outp = ctx.enter_context(tc.tile_pool(name="outp", bufs=3))
        ps_tr = ctx.enter_context(tc.tile_pool(name="ps_tr", bufs=2, space="PSUM"))
        ps_mm = ctx.enter_context(tc.tile_pool(name="ps_mm", bufs=2, space="PSUM"))

        # ---- constants (weights the PE needs first are DMA'd first) ----
        wv_s = consts.tile([128, DC, D], BF, tag="wv")
        wf_s = consts.tile([128, L, DC, D], BF, tag="wf")
        tb_s = consts.tile([128, H, NJ, 128], BF, tag="tb")
        i32 = consts.tile([128, 128], F32, tag="i32")
        ibf = consts.tile([128, 128], BF, tag="ibf")
        nc.sync.dma_start(ibf[:], idbf)
        nc.sync.dma_start(i32[:], id32)
        nc.sync.dma_start(wv_s[:], wv.rearrange("(c p) f -> p c f", p=128))
        epst = consts.tile([128, 1], F32, tag="eps")
        nc.vector.memset(epst[:], EPS)
        zerot = consts.tile([128, 1], F32, tag="zero")
        nc.vector.memset(zerot[:], 0.0)
        onet = consts.tile([128, 1], F32, tag="one")
        nc.vector.memset(onet[:], 1.0)
        halft = consts.tile([128, 1], F32, tag="half")
        nc.vector.memset(halft[:], 0.5)
        cvt = None
        if cv_nonzero:
            cvt = consts.tile([128, D], F32, tag="cv")
            nc.sync.dma_start(cvt[:], cv_d.to_broadcast((128, D)))
        bft = [None] * L
        lnfst = [None] * L
        lnfbt = [None] * L
        for l in range(L):
            if bf_nonzero[l]:
                bft[l] = consts.tile([128, D], F32, tag=f"bf{l}")
                nc.sync.dma_start(bft[l][:], bf_d[l].to_broadcast((128, D)))
            if lnf_uniform[l] is None:
                lnfst[l] = consts.tile([128, D], F32, tag=f"lnfs{l}")
                nc.sync.dma_start(lnfst[l][:], lnfs_d[l].to_broadcast((128, D)))
                lnfbt[l] = consts.tile([128, D], F32, tag=f"lnfb{l}")
                nc.sync.dma_start(lnfbt[l][:], lnfb_d[l].to_broadcast((128, D)))

        # ---- resident tensors ----
        X = xpool.tile([128, BPC, NJ, D], F32, tag="X")         # x, then x1
        V = vpool.tile([128, H, BPC, NJ, HS], BF, tag="V")      # per-head values

        I32i = mybir.dt.int32

        def emit_rsqrt(dst, var, uniq):
            """rstd = 1/sqrt(var + EPS) entirely on the DVE: fast-inverse-
            sqrt bit trick + 2 Newton iterations (rel err ~5e-6). Keeps the
            scalar engine's activation table free for Silu/Exp/Ln."""
            n = dst.shape[-1]
            ve = stat.tile([128, NT], F32, tag="rq_ve", name="rq_ve")[:, 0:n]
            hv = stat.tile([128, NT], F32, tag="rq_hv", name="rq_hv")[:, 0:n]
            ri = stat.tile([128, NT], I32i, tag="rq_ri", name="rq_ri")[:, 0:n]
            nc.vector.tensor_scalar(ve, var, EPS, None, op0=Alu.add)
            nc.vector.tensor_scalar(hv, ve, 0.5, None, op0=Alu.mult)
            nc.vector.tensor_scalar(ri, ve.bitcast(I32i), 1, None,
                                    op0=Alu.arith_shift_right)
            nc.vector.tensor_scalar(ri, ri, -1, 0x5f3759df,
                                    op0=Alu.mult, op1=Alu.add)
            cur = ri.bitcast(F32)
            for it in range(2):
                t1 = stat.tile([128, NT], F32, tag=f"rq_t{it}",
                               name=f"rq_t{it}")[:, 0:n]
                nc.vector.tensor_tensor(t1, cur, cur, op=Alu.mult)
                nc.vector.tensor_tensor(t1, t1, hv, op=Alu.mult)
                nc.vector.tensor_scalar(t1, t1, -1.0, 1.5,
                                        op0=Alu.mult, op1=Alu.add)
                out = dst if it == 1 else stat.tile(
                    [128, NT], F32, tag="rq_c", name="rq_c")[:, 0:n]
                nc.vector.tensor_tensor(out, cur, t1, op=Alu.mult)
                cur = out

        # ================= phase A: LN + v-projection =================
        # Per batch: 8 chunks in sub-batches of 4 (stats -> packed DVE rsqrt
        # -> normalize on ACT + transpose + project), then phase B (circulant
        # + residual) for that batch so the pipeline stays at half depth.
        mvA = statp.tile([128, NT, 2], F32, tag="mvA")
        rsA = statp.tile([128, NT], F32, tag="rsA")
        nbA = statp.tile([128, NT], F32, tag="nbA")
        AB = 4
        for t0 in range(0, NT, AB):
            for t in range(t0, t0 + AB):
                b, jc = divmod(t, NJ)
                xt = X[:, b, jc, :]
                nc.sync.dma_start(xt, xs[b, jc * 128:(jc + 1) * 128, :])
                st = stat.tile([128, 3, 6], F32, tag="bst")
                for g in range(3):
                    nc.vector.bn_stats(st[:, g, :], xt[:, g * 256:(g + 1) * 256])
                nc.vector.bn_aggr(mvA[:, t, :], st[:])
            emit_rsqrt(rsA[:, t0:t0 + AB], mvA[:, t0:t0 + AB, 1], t0)
            # per-token -mu*rstd for the ACT-engine normalize, one batched op
            nc.vector.scalar_tensor_tensor(
                nbA[:, t0:t0 + AB], mvA[:, t0:t0 + AB, 0], -1.0,
                rsA[:, t0:t0 + AB], op0=Alu.mult, op1=Alu.mult)
            for t in range(t0, t0 + AB):
                b, jc = divmod(t, NJ)
                xt = X[:, b, jc, :]
                u = upool.tile([128, D], BF, tag="u")
                nc.scalar.activation(u[:], xt, Act.Identity,
                                     bias=nbA[:, t:t + 1],
                                     scale=rsA[:, t:t + 1])
                ptr = ps_tr.tile([128, D], BF, tag="tr")
                for c in range(DC):
                    nc.tensor.transpose(ptr[:, c * 128:(c + 1) * 128],
                                        u[:, c * 128:(c + 1) * 128], ibf[:])
                udt = dtp.tile([128, D], BF, tag="udt")
                nc.scalar.copy(udt[:], ptr[:])
                pv = ps_mm.tile([128, D], F32, tag="mm")
                for c in range(DC):
                    nc.tensor.matmul(pv[:, 0:512], udt[:, c * 128:(c + 1) * 128],
                                     wv_s[:, c, 0:512],
                                     start=(c == 0), stop=(c == DC - 1))
                    nc.tensor.matmul(pv[:, 512:D], udt[:, c * 128:(c + 1) * 128],
                                     wv_s[:, c, 512:D],
                                     start=(c == 0), stop=(c == DC - 1))
                vdst = V[:, :, b, jc, :]                             # [128, H, HS]
                pv3 = pv[:].rearrange("p (h k) -> p h k", h=H)
                if cv_nonzero:
                    cv3 = cvt[:].rearrange("p (h k) -> p h k", h=H)
                    nc.vector.tensor_tensor(vdst, pv3, cv3, op=Alu.add)
                elif t % 2 == 0:
                    # alternate the PSUM->SBUF copy between DVE and ACT so
                    # neither engine paces the PE during phase A
                    nc.vector.tensor_copy(vdst, pv3)
                else:
                    nc.scalar.copy(vdst, pv3)
            if t0 // AB == 1:
                # Toeplitz bank + FFN weights: clock-gate these bulk loads so
                # they don't steal HBM bandwidth from the stats-critical x
                # chunk loads during the ramp; issue off the Pool sequencer
                # to keep the SP queue short.
                with tc.tile_wait_until(0.018):
                    for h in range(H):
                        nc.gpsimd.dma_start(
                            tb_s[:, h, :, :],
                            tb_d[h].rearrange("p (m f) -> p m f", m=NJ))
                with tc.tile_wait_until(0.028):
                    nc.gpsimd.dma_start(
                        wf_s[:, 0, :, :],
                        wf[0].rearrange("(c p) f -> p c f", p=128))
                with tc.tile_wait_until(0.040):
                    nc.gpsimd.dma_start(
                        wf_s[:, 1, :, :],
                        wf[1].rearrange("(c p) f -> p c f", p=128))
            # ======== phase B for batch bb once its 8 chunks are in V ======
            # y[ic] = sum_m T[m] @ v[(ic+m)%8]: for each diagonal m the ic
            # range splits into two contiguous runs, so the moving operand is
            # up to 512 wide and the PE streams instead of reloading weights.
            if (t0 + AB) % NJ == 0:
                bb = (t0 + AB) // NJ - 1
                for h in range(H):
                    pc = ps_mm.tile([128, NJ, HS], F32, tag="mm")
                    for m in range(NJ):
                        k = NJ - m
                        nc.tensor.matmul(pc[:, 0:k, :], tb_s[:, h, m, :],
                                         V[:, h, bb, m:NJ, :],
                                         start=(m == 0), stop=(m == NJ - 1),
                                         skip_group_check=True)
                        if m > 0:
                            nc.tensor.matmul(pc[:, k:NJ, :], tb_s[:, h, m, :],
                                             V[:, h, bb, 0:m, :],
                                             start=False, stop=(m == NJ - 1),
                                             skip_group_check=True)
                    xap = X[:, bb, :, h * HS:(h + 1) * HS]           # [128,NJ,HS]
                    nc.vector.tensor_tensor(xap, xap, pc[:], op=Alu.add)

        # ================= phase C: FFN x2 + log_cosh =================
        # Token-major FFN: per chunk, PE-transpose the input, Dense matmul
        # into PSUM, copy+row-sum to SBUF, sumsq on the bf16 copy, batched
        # DVE rsqrt, then Silu with per-token scale/bias. Layer 2 runs in
        # groups so each group's log_cosh tail overlaps the next group's
        # matmuls; exp/ln are batched per group (they live in one activation
        # table, but separate tables from Silu).
        inv_d = 1.0 / D
        zcur = [None] * NT
        sums = [statp.tile([128, NT], F32, tag=f"sum{l}", name=f"sum{l}")
                for l in range(L)]
        ssq = [statp.tile([128, NT], F32, tag=f"ssq{l}", name=f"ssq{l}")
               for l in range(L)]
        rsF = [statp.tile([128, NT], F32, tag=f"rs{l}", name=f"rs{l}")
               for l in range(L)]
        biasF = [statp.tile([128, NT], F32, tag=f"bi{l}", name=f"bi{l}")
                 for l in range(L)]

        def ffn_chunks(l, ts, y_on_dve, zdt_on_scalar=False):
            """Dense matmuls + sum/sumsq accumulation for chunks ts of
            layer l. y_on_dve routes the PSUM->SBUF copy to the DVE so the
            scalar engine stays free for the previous group's tail."""
            for t in ts:
                b, jc = divmod(t, NJ)
                src_ = X[:, b, jc, :] if l == 0 else zcur[t][:]
                ptr = ps_tr.tile([128, D], F32 if l == 0 else BF, tag="tr")
                ident = i32 if l == 0 else ibf
                for c in range(DC):
                    nc.tensor.transpose(ptr[:, c * 128:(c + 1) * 128],
                                        src_[:, c * 128:(c + 1) * 128], ident[:])
                zdt = dtp.tile([128, D], BF, tag="zdt")
                if zdt_on_scalar:
                    nc.scalar.copy(zdt[:], ptr[:])
                else:
                    nc.vector.tensor_copy(zdt[:], ptr[:])
                pf = ps_mm.tile([128, D], F32, tag="mm")
                for c in range(DC):
                    nc.tensor.matmul(pf[:, 0:512], zdt[:, c * 128:(c + 1) * 128],
                                     wf_s[:, l, c, 0:512],
                                     start=(c == 0), stop=(c == DC - 1))
                    nc.tensor.matmul(pf[:, 512:D], zdt[:, c * 128:(c + 1) * 128],
                                     wf_s[:, l, c, 512:D],
                                     start=(c == 0), stop=(c == DC - 1))
                if bf_nonzero[l]:
                    nc.vector.tensor_tensor(pf[:], pf[:], bft[l][:], op=Alu.add)
                y = acts.tile([128, D], BF, tag="acts")
                if y_on_dve:
                    nc.vector.tensor_scalar(y[:], pf[:], 1.0, 0.0, op0=Alu.mult,
                                            op1=Alu.add,
                                            accum_out=sums[l][:, t:t + 1])
                else:
                    nc.scalar.activation(y[:], pf[:], Act.Copy,
                                         accum_out=sums[l][:, t:t + 1])
                scr = wkp.tile([128, D], BF, tag="scr")
                nc.vector.scalar_tensor_tensor(
                    scr[:], y[:], 0.0, y[:], op0=Alu.add, op1=Alu.mult,
                    accum_out=ssq[l][:, t:t + 1])
                zcur[t] = y

        def ffn_epilogue(l, t0, t1):
            """Batched LN stats -> DVE rstd/bias -> silu for chunks [t0, t1)."""
            sl = slice(t0, t1)
            mu = stat.tile([128, NT], F32, tag="muE")
            var = stat.tile([128, NT], F32, tag="varE")
            nc.vector.tensor_scalar(mu[:, sl], sums[l][:, sl], inv_d, None,
                                    op0=Alu.mult)
            nc.vector.scalar_tensor_tensor(var[:, sl], mu[:, sl], -1.0,
                                           mu[:, sl], op0=Alu.mult, op1=Alu.mult)
            nc.vector.scalar_tensor_tensor(var[:, sl], ssq[l][:, sl], inv_d,
                                           var[:, sl], op0=Alu.mult, op1=Alu.add)
            emit_rsqrt(rsF[l][:, sl], var[:, sl], 100 + l * 16 + t0)
            fast = lnf_uniform[l] is not None
            if fast:
                cs, cb = lnf_uniform[l]
                if cs != 1.0:
                    nc.vector.tensor_scalar(rsF[l][:, sl], rsF[l][:, sl],
                                            float(cs), None, op0=Alu.mult)
                nc.vector.scalar_tensor_tensor(biasF[l][:, sl], mu[:, sl], -1.0,
                                               rsF[l][:, sl],
                                               op0=Alu.mult, op1=Alu.mult)
                if cb != 0.0:
                    nc.vector.tensor_scalar(biasF[l][:, sl], biasF[l][:, sl],
                                            float(cb), None, op0=Alu.add)
                for t in range(t0, t1):
                    y = zcur[t]
                    nc.scalar.activation(y[:], y[:], Act.Silu,
                                         bias=biasF[l][:, t:t + 1],
                                         scale=rsF[l][:, t:t + 1])
            else:
                for t in range(t0, t1):
                    y = zcur[t]
                    tmp = acts.tile([128, D], BF, tag="acts")
                    nc.vector.tensor_scalar(tmp[:], y[:], mu[:, t:t + 1],
                                            rsF[l][:, t:t + 1],
                                            op0=Alu.subtract, op1=Alu.mult)
                    nc.vector.tensor_tensor(tmp[:], tmp[:], lnfst[l][:],
                                            op=Alu.mult)
                    nc.vector.tensor_tensor(tmp[:], tmp[:], lnfbt[l][:],
                                            op=Alu.add)
                    nc.scalar.activation(tmp[:], tmp[:], Act.Silu,
                                         bias=zerot[:])
                    zcur[t] = tmp

        def runs_of(ts):
            """Split chunk ids into runs contiguous in X's (b, jc) layout."""
            out = []
            for t in ts:
                if out and t == out[-1][-1] + 1 and (t % NJ) != 0:
                    out[-1].append(t)
                else:
                    out.append([t])
            return out

        def tail(ts):
            """log_cosh(w) = w + [log1p(exp(-2w)) - log2], w = z2 + x1 (in X).
            No abs needed: the identity holds for both signs and exp(-2w)
            stays <= ~e^18 here. -log2 folds into the ln via
            ln(0.5*e + 0.5) = log1p(e) - log2. w-adds run on the Pool
            engine; exp/ln batch over contiguous runs; final adds alternate
            DVE/Pool and the group's output leaves as one DMA."""
            ts = list(ts)
            k = len(ts)
            for t in ts:
                b, jc = divmod(t, NJ)
                nc.vector.tensor_tensor(X[:, b, jc, :], X[:, b, jc, :],
                                        zcur[t][:], op=Alu.add)
            sp = spp.tile([128, 6, D], BF, tag="sp")
            for run in runs_of(ts):
                b, jc = divmod(run[0], NJ)
                i = run[0] - ts[0]
                nc.scalar.activation(sp[:, i:i + len(run), :],
                                     X[:, b, jc:jc + len(run), :], Act.Exp,
                                     bias=zerot[:], scale=-2.0)
            nc.scalar.activation(sp[:, 0:k, :], sp[:, 0:k, :], Act.Ln,
                                 bias=halft[:], scale=0.5)
            for t in ts:
                b, jc = divmod(t, NJ)
                i = t - ts[0]
                ot = outp.tile([128, D], F32, tag="ot")
                eng = nc.gpsimd if t % 2 == 0 else nc.vector
                eng.tensor_tensor(ot[:], X[:, b, jc, :], sp[:, i, :],
                                  op=Alu.add)
                nc.sync.dma_start(out_d[b, jc * 128:(jc + 1) * 128, :], ot[:])

        # layer 1 in halves: the first half's silu lands while the PE is
        # still on the second half, so layer 2 starts without a bubble. The
        # second half's PSUM drains run on the DVE — on the in-order scalar
        # queue they would sit behind the first half's silu batch and stall
        # the PE on PSUM buffers.
        ffn_chunks(0, range(0, NT // 2), y_on_dve=False, zdt_on_scalar=True)
        ffn_epilogue(0, 0, NT // 2)
        ffn_chunks(0, range(NT // 2, NT), y_on_dve=True)
        ffn_epilogue(0, NT // 2, NT)
        # layer 2 in groups: each group's tail is emitted before the NEXT
        # group's epilogue, so it overlaps those matmuls and never waits on
        # the last silu batch.
        GROUPS = [range(0, 6), range(6, 12), range(12, 16)]
        for gi, g in enumerate(GROUPS):
            ffn_chunks(1, g, y_on_dve=(gi > 0))
            if gi > 0:
                tail(GROUPS[gi - 1])
            ffn_epilogue(1, g.start, g.stop)
        tail(GROUPS[-1])

def _prep(inputs):
    x = np.asarray(inputs["x"], np.float32)
    ln1_s = np.asarray(inputs["ln1_scale"], np.float32)
    ln1_b = np.asarray(inputs["ln1_bias"], np.float32)
    Wv = np.asarray(inputs["Wv"], np.float32)
    alpha = np.asarray(inputs["alpha"], np.float32)
    Wf = np.asarray(inputs["Wf"], np.float32)
    bfv = np.asarray(inputs["bf"], np.float32)
    lnf_s = np.asarray(inputs["lnf_scale"], np.float32)
    lnf_b = np.asarray(inputs["lnf_bias"], np.float32)

    Wv_flat = Wv.transpose(1, 0, 2).reshape(D, H * HS)
    Wvp = (ln1_s[:, None] * Wv_flat).astype(BF16)
    cv = (ln1_b @ Wv_flat).astype(np.float32)

    ar = alpha[:, (-np.arange(N)) % N]
    ar2 = np.concatenate([ar, ar], axis=1)
    m_ = np.arange(NJ)[:, None, None]
    p_ = np.arange(128)[None, :, None]
    f_ = np.arange(128)[None, None, :]
    T = ar2[:, N + 128 * m_ + p_ - f_]                  # [H, NJ, 128, 128]
    tbank = np.ascontiguousarray(
        T.transpose(0, 2, 1, 3).reshape(H, 128, NJ * 128)).astype(BF16)

    cv_nonzero = bool(np.any(cv))
    bf_nonzero = tuple(bool(np.any(bfv[l])) for l in range(L))
    lnf_uniform = []
    for l in range(L):
        s, bb = lnf_s[l], lnf_b[l]
        if np.all(s == s[0]) and np.all(bb == bb[0]):
            lnf_uniform.append((float(s[0]), float(bb[0])))
        else:
            lnf_uniform.append(None)
    key = (cv_nonzero, bf_nonzero, tuple(lnf_uniform))

    common = {
        "wv": np.ascontiguousarray(Wvp),
        "wf": Wf.astype(BF16),
        "tbank": tbank,
        "id32": np.eye(128, dtype=np.float32),
        "idbf": np.eye(128, dtype=BF16),
        "cv": cv,
        "bfb": bfv,
        "lnfs": lnf_s,
        "lnfb": lnf_b,
    }
    return x, key, common, (cv_nonzero, bf_nonzero, lnf_uniform)


def kernel(**inputs):
    x, key, common, flags = _prep(inputs)
    if key not in _cache:
        _cache[key] = _build(*flags)
    nc = _cache[key]
    in_maps = []
    for i in range(NCORES):
        m = dict(common)
        m["xs"] = np.ascontiguousarray(x[i * BPC:(i + 1) * BPC])
        in_maps.append(m)
    res = run_bass_kernel_spmd(nc, in_maps, core_ids=list(range(NCORES)),
                               trace=TRACE, **TRACE_KW)
    kernel.last_result = res
    out = np.empty((B, N, D), np.float32)
    for i in range(NCORES):
        out[i * BPC:(i + 1) * BPC] = res.results[i]["out"]
    return out


# revision 29
# speedup vs baseline: 1.0842x; 1.0842x over previous
"""Trainium2 Bass kernel for nn_CoreBlock (circulant attention + 2-layer FFN).

Contract: kernel(**inputs) takes FULL unsharded inputs (as produced by
setup_inputs) and returns the FULL [16, 1024, 768] f32 output.

Strategy: pure data-parallel over batch — 8 NeuronCores x 2 batches each.
All weights replicated. Per core:
  phase A: LayerNorm(x) -> u (stats on DVE, rstd via a DVE Newton rsqrt,
           normalize on the ACT engine with per-token scale/bias),
           PE-transpose u, v = u_dt.T @ Wv' per token chunk, gathered into
           a resident V tensor [128, H, B, NJ, HS] in SBUF. The Toeplitz
           bank / FFN weight DMAs are queued behind batch 0's x chunks so
           the ramp-critical loads get the HBM bandwidth first.
  phase B: per head h and batch b: circulant matmul y[ic] = sum_m T[m] @
           v[(ic+m)%8]; each diagonal m is 2 wide matmuls (moving free up
           to 512) so the PE streams instead of reloading weights per
           64-wide product. Residual-added into X (X becomes x1).
  phase C: 2x [Dense -> LayerNorm -> swish], token-major with PE
           transposes between layers. Layer 1 runs in halves, layer 2 in
           groups of [7, 6, 3]: each group's log_cosh tail overlaps the
           next group's matmuls. Groups after the first route their PSUM
           drains to the DVE so the in-order scalar queue can keep draining
           the previous tail. Tail: log_cosh(w) = w + [log1p(exp(-2w)) -
           log2] with -log2 folded into the ln (ln(0.5e + 0.5)); exp ops
           and ln ops batch separately (the runtime assigns them different
           activation tables); w-adds and final adds alternate DVE/Pool.

Matmul operands are bf16 (full-rate PE, fp32 PSUM accumulation); stats and
elementwise math are fp32. The scalar engine's activation tables only ever
cycle between Silu and Exp/Ln (sqrt lives on the DVE), keeping table
reloads off the critical path.
"""

import math
import numpy as np
import ml_dtypes

import concourse.bass as bass
import concourse.tile as tile
from concourse import bacc, mybir
from concourse.bass_utils import run_bass_kernel_spmd

BF16 = ml_dtypes.bfloat16

B, N, D = 16, 1024, 768
H, HS, L = 12, 64, 2
EPS = 1e-6
NCORES = 8
BPC = B // NCORES          # batches per core
NJ = N // 128              # token chunks per batch (8)
NT = BPC * NJ              # token chunks per core (16)
DC = D // 128              # feature chunks (6)

F32 = mybir.dt.float32
BF = mybir.dt.bfloat16
Alu = mybir.AluOpType
Act = mybir.ActivationFunctionType

TRACE = False              # test harness sets this for profiling runs
TRACE_KW = {}

_cache = {}


def _build(cv_nonzero, bf_nonzero, lnf_uniform):
    """Construct the per-core Bass program. lnf_uniform: per-layer (cs, cb)
    if lnf scale/bias are uniform scalars, else None entries."""
    nc = bacc.Bacc("TRN2", target_bir_lowering=False, debug=False)

    xs = nc.dram_tensor("xs", (BPC, N, D), F32, kind="ExternalInput").ap()
    wv = nc.dram_tensor("wv", (D, D), BF, kind="ExternalInput").ap()
    wf = nc.dram_tensor("wf", (L, D, D), BF, kind="ExternalInput").ap()
    tb_d = nc.dram_tensor("tbank", (H, 128, NJ * 128), BF, kind="ExternalInput").ap()
    id32 = nc.dram_tensor("id32", (128, 128), F32, kind="ExternalInput").ap()
    idbf = nc.dram_tensor("idbf", (128, 128), BF, kind="ExternalInput").ap()
    cv_d = nc.dram_tensor("cv", (D,), F32, kind="ExternalInput").ap()
    bf_d = nc.dram_tensor("bfb", (L, D), F32, kind="ExternalInput").ap()
    lnfs_d = nc.dram_tensor("lnfs", (L, D), F32, kind="ExternalInput").ap()
    lnfb_d = nc.dram_tensor("lnfb", (L, D), F32, kind="ExternalInput").ap()
    out_d = nc.dram_tensor("out", (BPC, N, D), F32, kind="ExternalOutput").ap()

    with tile.TileContext(nc) as tc:
        _emit(nc, tc, xs, wv, wf, tb_d, id32, idbf, cv_d, bf_d, lnfs_d, lnfb_d,
              out_d, cv_nonzero, bf_nonzero, lnf_uniform)
    nc.compile()
    return nc


def _emit(nc, tc, xs, wv, wf, tb_d, id32, idbf, cv_d, bf_d, lnfs_d, lnfb_d,
          out_d, cv_nonzero, bf_nonzero, lnf_uniform):
    from contextlib import ExitStack
    ctx = ExitStack()
    with ctx:
        consts = ctx.enter_context(tc.tile_pool(name="consts", bufs=1))
        xpool = ctx.enter_context(tc.tile_pool(name="xpool", bufs=1))
        vpool = ctx.enter_context(tc.tile_pool(name="vpool", bufs=1))
        acts = ctx.enter_context(tc.tile_pool(name="acts", bufs=18))
        upool = ctx.enter_context(tc.tile_pool(name="upool", bufs=3))
        dtp = ctx.enter_context(tc.tile_pool(name="dtp", bufs=3))
        stat = ctx.enter_context(tc.tile_pool(name="stat", bufs=4))
        statp = ctx.enter_context(tc.tile_pool(name="statp", bufs=2))
        wkp = ctx.enter_context(tc.tile_pool(name="wkp", bufs=3))
        spp = ctx.enter_context(tc.tile_pool(name="spp", bufs=2))
        outp = ctx.enter_context(tc.tile_pool(name="outp", bufs=3))
        ps_tr = ctx.enter_context(tc.tile_pool(name="ps_tr", bufs=2, space="PSUM"))
        ps_mm = ctx.enter_context(tc.tile_pool(name="ps_mm", bufs=2, space="PSUM"))

        # ---- constants (weights the PE needs first are DMA'd first) ----
        wv_s = consts.tile([128, DC, D], BF, tag="wv")
        wf_s = consts.tile([128, L, DC, D], BF, tag="wf")
        tb_s = consts.tile([128, H, NJ, 128], BF, tag="tb")
        i32 = consts.tile([128, 128], F32, tag="i32")
        ibf = consts.tile([128, 128], BF, tag="ibf")
        nc.sync.dma_start(ibf[:], idbf)
        nc.sync.dma_start(i32[:], id32)
        nc.sync.dma_start(wv_s[:], wv.rearrange("(c p) f -> p c f", p=128))
        epst = consts.tile([128, 1], F32, tag="eps")
        nc.vector.memset(epst[:], EPS)
        zerot = consts.tile([128, 1], F32, tag="zero")
        nc.vector.memset(zerot[:], 0.0)
        onet = consts.tile([128, 1], F32, tag="one")
        nc.vector.memset(onet[:], 1.0)
        halft = consts.tile([128, 1], F32, tag="half")
        nc.vector.memset(halft[:], 0.5)
        cvt = None
        if cv_nonzero:
            cvt = consts.tile([128, D], F32, tag="cv")
            nc.sync.dma_start(cvt[:], cv_d.to_broadcast((128, D)))
        bft = [None] * L
        lnfst = [None] * L
        lnfbt = [None] * L
        for l in range(L):
            if bf_nonzero[l]:
                bft[l] = consts.tile([128, D], F32, tag=f"bf{l}")
                nc.sync.dma_start(bft[l][:], bf_d[l].to_broadcast((128, D)))
            if lnf_uniform[l] is None:
                lnfst[l] = consts.tile([128, D], F32, tag=f"lnfs{l}")
                nc.sync.dma_start(lnfst[l][:], lnfs_d[l].to_broadcast((128, D)))
                lnfbt[l] = consts.tile([128, D], F32, tag=f"lnfb{l}")
                nc.sync.dma_start(lnfbt[l][:], lnfb_d[l].to_broadcast((128, D)))

        # ---- resident tensors ----
        X = xpool.tile([128, BPC, NJ, D], F32, tag="X")         # x, then x1
        V = vpool.tile([128, H, BPC, NJ, HS], BF, tag="V")      # per-head values

        I32i = mybir.dt.int32

        def emit_rsqrt(dst, var, uniq):
            """rstd = 1/sqrt(var + EPS) entirely on the DVE: fast-inverse-
            sqrt bit trick + 2 Newton iterations (rel err ~5e-6). Keeps the
            scalar engine's activation table free for Silu/Exp/Ln."""
            n = dst.shape[-1]
            ve = stat.tile([128, NT], F32, tag="rq_ve", name="rq_ve")[:, 0:n]
            hv = stat.tile([128, NT], F32, tag="rq_hv", name="rq_hv")[:, 0:n]
            ri = stat.tile([128, NT], I32i, tag="rq_ri", name="rq_ri")[:, 0:n]
            nc.vector.tensor_scalar(ve, var, EPS, None, op0=Alu.add)
            nc.vector.tensor_scalar(hv, ve, 0.5, None, op0=Alu.mult)
            nc.vector.tensor_scalar(ri, ve.bitcast(I32i), 1, None,
                                    op0=Alu.arith_shift_right)
            nc.vector.tensor_scalar(ri, ri, -1, 0x5f3759df,
                                    op0=Alu.mult, op1=Alu.add)
            cur = ri.bitcast(F32)
            for it in range(2):
                t1 = stat.tile([128, NT], F32, tag=f"rq_t{it}",
                               name=f"rq_t{it}")[:, 0:n]
                nc.vector.tensor_tensor(t1, cur, cur, op=Alu.mult)
                nc.vector.tensor_tensor(t1, t1, hv, op=Alu.mult)
                nc.vector.tensor_scalar(t1, t1, -1.0, 1.5,
                                        op0=Alu.mult, op1=Alu.add)
                out = dst if it == 1 else stat.tile(
                    [128, NT], F32, tag="rq_c", name="rq_c")[:, 0:n]
                nc.vector.tensor_tensor(out, cur, t1, op=Alu.mult)
                cur = out

        # ================= phase A: LN + v-projection =================
        # Per batch: 8 chunks in sub-batches of 4 (stats -> packed DVE rsqrt
        # -> normalize on ACT + transpose + project), then phase B (circulant
        # + residual) for that batch so the pipeline stays at half depth.
        mvA = statp.tile([128, NT, 2], F32, tag="mvA")
        rsA = statp.tile([128, NT], F32, tag="rsA")
        nbA = statp.tile([128, NT], F32, tag="nbA")
        AB = 4
        for t0 in range(0, NT, AB):
            for t in range(t0, t0 + AB):
                b, jc = divmod(t, NJ)
                xt = X[:, b, jc, :]
                nc.sync.dma_start(xt, xs[b, jc * 128:(jc + 1) * 128, :])
                st = stat.tile([128, 3, 6], F32, tag="bst")
                for g in range(3):
                    nc.vector.bn_stats(st[:, g, :], xt[:, g * 256:(g + 1) * 256])
                nc.vector.bn_aggr(mvA[:, t, :], st[:])
            emit_rsqrt(rsA[:, t0:t0 + AB], mvA[:, t0:t0 + AB, 1], t0)
            # per-token -mu*rstd for the ACT-engine normalize, one batched op
            nc.vector.scalar_tensor_tensor(
                nbA[:, t0:t0 + AB], mvA[:, t0:t0 + AB, 0], -1.0,
                rsA[:, t0:t0 + AB], op0=Alu.mult, op1=Alu.mult)
            from contextlib import nullcontext
            prio = tc.high_priority() if t0 == 0 else nullcontext()
            with prio:
                for t in range(t0, t0 + AB):
                    b, jc = divmod(t, NJ)
                    xt = X[:, b, jc, :]
                    u = upool.tile([128, D], BF, tag="u")
                    nc.scalar.activation(u[:], xt, Act.Identity,
                                         bias=nbA[:, t:t + 1],
                                         scale=rsA[:, t:t + 1])
                    ptr = ps_tr.tile([128, D], BF, tag="tr")
                    for c in range(DC):
                        nc.tensor.transpose(ptr[:, c * 128:(c + 1) * 128],
                                            u[:, c * 128:(c + 1) * 128], ibf[:])
                    udt = dtp.tile([128, D], BF, tag="udt")
                    nc.scalar.copy(udt[:], ptr[:])
                    pv = ps_mm.tile([128, D], F32, tag="mm")
                    for c in range(DC):
                        nc.tensor.matmul(pv[:, 0:512],
                                         udt[:, c * 128:(c + 1) * 128],
                                         wv_s[:, c, 0:512],
                                         start=(c == 0), stop=(c == DC - 1))
                        nc.tensor.matmul(pv[:, 512:D],
                                         udt[:, c * 128:(c + 1) * 128],
                                         wv_s[:, c, 512:D],
                                         start=(c == 0), stop=(c == DC - 1))
                    vdst = V[:, :, b, jc, :]                         # [128, H, HS]
                    pv3 = pv[:].rearrange("p (h k) -> p h k", h=H)
                    if cv_nonzero:
                        cv3 = cvt[:].rearrange("p (h k) -> p h k", h=H)
                        nc.vector.tensor_tensor(vdst, pv3, cv3, op=Alu.add)
                    elif t % 2 == 0:
                        # alternate the PSUM->SBUF copy between DVE and ACT
                        nc.vector.tensor_copy(vdst, pv3)
                    else:
                        nc.scalar.copy(vdst, pv3)
            if t0 // AB == 1:
                # Toeplitz bank on the SP queue AFTER batch 0's x-chunk
                # issues: the SP issues in order, so the bank transfer can't
                # steal HBM bandwidth from the stats-critical first loads.
                for h in range(H):
                    nc.sync.dma_start(
                        tb_s[:, h, :, :],
                        tb_d[h].rearrange("p (m f) -> p m f", m=NJ))
            if t0 // AB == 2:
                nc.sync.dma_start(
                    wf_s[:, 0, :, :], wf[0].rearrange("(c p) f -> p c f", p=128))
            if t0 // AB == 3:
                nc.sync.dma_start(
                    wf_s[:, 1, :, :], wf[1].rearrange("(c p) f -> p c f", p=128))
            # ======== phase B for batch bb once its 8 chunks are in V ======
            # y[ic] = sum_m T[m] @ v[(ic+m)%8]: for each diagonal m the ic
            # range splits into two contiguous runs, so the moving operand is
            # up to 512 wide and the PE streams instead of reloading weights.
            if (t0 + AB) % NJ == 0:
                bb = (t0 + AB) // NJ - 1
                for h in range(H):
                    pc = ps_mm.tile([128, NJ, HS], F32, tag="mm")
                    for m in range(NJ):
                        k = NJ - m
                        nc.tensor.matmul(pc[:, 0:k, :], tb_s[:, h, m, :],
                                         V[:, h, bb, m:NJ, :],
                                         start=(m == 0), stop=(m == NJ - 1),
                                         skip_group_check=True)
                        if m > 0:
                            nc.tensor.matmul(pc[:, k:NJ, :], tb_s[:, h, m, :],
                                             V[:, h, bb, 0:m, :],
                                             start=False, stop=(m == NJ - 1),
                                             skip_group_check=True)
                    xap = X[:, bb, :, h * HS:(h + 1) * HS]           # [128,NJ,HS]
                    nc.vector.tensor_tensor(xap, xap, pc[:], op=Alu.add)

        # ================= phase C: FFN x2 + log_cosh =================
        # Token-major FFN: per chunk, PE-transpose the input, Dense matmul
        # into PSUM, copy+row-sum to SBUF, sumsq on the bf16 copy, batched
        # DVE rsqrt, then Silu with per-token scale/bias. Layer 2 runs in
        # groups so each group's log_cosh tail overlaps the next group's
        # matmuls; exp/ln are batched per group (they live in one activation
        # table, but separate tables from Silu).
        inv_d = 1.0 / D
        zcur = [None] * NT
        sums = [statp.tile([128, NT], F32, tag=f"sum{l}", name=f"sum{l}")
                for l in range(L)]
        ssq = [statp.tile([128, NT], F32, tag=f"ssq{l}", name=f"ssq{l}")
               for l in range(L)]
        rsF = [statp.tile([128, NT], F32, tag=f"rs{l}", name=f"rs{l}")
               for l in range(L)]
        biasF = [statp.tile([128, NT], F32, tag=f"bi{l}", name=f"bi{l}")
                 for l in range(L)]

        def ffn_chunks(l, ts, y_on_dve, zdt_on_scalar=False):
            """Dense matmuls + sum/sumsq accumulation for chunks ts of
            layer l. y_on_dve routes the PSUM->SBUF copy to the DVE so the
            scalar engine stays free for the previous group's tail."""
            for t in ts:
                b, jc = divmod(t, NJ)
                src_ = X[:, b, jc, :] if l == 0 else zcur[t][:]
                ptr = ps_tr.tile([128, D], F32 if l == 0 else BF, tag="tr")
                ident = i32 if l == 0 else ibf
                for c in range(DC):
                    nc.tensor.transpose(ptr[:, c * 128:(c + 1) * 128],
                                        src_[:, c * 128:(c + 1) * 128], ident[:])
                zdt = dtp.tile([128, D], BF, tag="zdt")
                if zdt_on_scalar:
                    nc.scalar.copy(zdt[:], ptr[:])
                else:
                    nc.vector.tensor_copy(zdt[:], ptr[:])
                pf = ps_mm.tile([128, D], F32, tag="mm")
                for c in range(DC):
                    nc.tensor.matmul(pf[:, 0:512], zdt[:, c * 128:(c + 1) * 128],
                                     wf_s[:, l, c, 0:512],
                                     start=(c == 0), stop=(c == DC - 1))
                    nc.tensor.matmul(pf[:, 512:D], zdt[:, c * 128:(c + 1) * 128],
                                     wf_s[:, l, c, 512:D],
                                     start=(c == 0), stop=(c == DC - 1))
                if bf_nonzero[l]:
                    nc.vector.tensor_tensor(pf[:], pf[:], bft[l][:], op=Alu.add)
                y = acts.tile([128, D], BF, tag="acts")
                if y_on_dve:
                    nc.vector.tensor_scalar(y[:], pf[:], 1.0, 0.0, op0=Alu.mult,
                                            op1=Alu.add,
                                            accum_out=sums[l][:, t:t + 1])
                else:
                    nc.scalar.activation(y[:], pf[:], Act.Copy,
                                         accum_out=sums[l][:, t:t + 1])
                scr = wkp.tile([128, D], BF, tag="scr")
                nc.vector.scalar_tensor_tensor(
                    scr[:], y[:], 0.0, y[:], op0=Alu.add, op1=Alu.mult,
                    accum_out=ssq[l][:, t:t + 1])
                zcur[t] = y

        def ffn_epilogue(l, t0, t1):
            """Batched LN stats -> DVE rstd/bias -> silu for chunks [t0, t1)."""
            sl = slice(t0, t1)
            mu = stat.tile([128, NT], F32, tag="muE")
            var = stat.tile([128, NT], F32, tag="varE")
            nc.vector.tensor_scalar(mu[:, sl], sums[l][:, sl], inv_d, None,
                                    op0=Alu.mult)
            nc.vector.scalar_tensor_tensor(var[:, sl], mu[:, sl], -1.0,
                                           mu[:, sl], op0=Alu.mult, op1=Alu.mult)
            nc.vector.scalar_tensor_tensor(var[:, sl], ssq[l][:, sl], inv_d,
                                           var[:, sl], op0=Alu.mult, op1=Alu.add)
            emit_rsqrt(rsF[l][:, sl], var[:, sl], 100 + l * 16 + t0)
            fast = lnf_uniform[l] is not None
            if fast:
                cs, cb = lnf_uniform[l]
                if cs != 1.0:
                    nc.vector.tensor_scalar(rsF[l][:, sl], rsF[l][:, sl],
                                            float(cs), None, op0=Alu.mult)
                nc.vector.scalar_tensor_tensor(biasF[l][:, sl], mu[:, sl], -1.0,
                                               rsF[l][:, sl],
                                               op0=Alu.mult, op1=Alu.mult)
                if cb != 0.0:
                    nc.vector.tensor_scalar(biasF[l][:, sl], biasF[l][:, sl],
                                            float(cb), None, op0=Alu.add)
                for t in range(t0, t1):
                    y = zcur[t]
                    nc.scalar.activation(y[:], y[:], Act.Silu,
                                         bias=biasF[l][:, t:t + 1],
                                         scale=rsF[l][:, t:t + 1])
            else:
                for t in range(t0, t1):
                    y = zcur[t]
                    tmp = acts.tile([128, D], BF, tag="acts")
                    nc.vector.tensor_scalar(tmp[:], y[:], mu[:, t:t + 1],
                                            rsF[l][:, t:t + 1],
                                            op0=Alu.subtract, op1=Alu.mult)
                    nc.vector.tensor_tensor(tmp[:], tmp[:], lnfst[l][:],
                                            op=Alu.mult)
                    nc.vector.tensor_tensor(tmp[:], tmp[:], lnfbt[l][:],
                                            op=Alu.add)
                    nc.scalar.activation(tmp[:], tmp[:], Act.Silu,
                                         bias=zerot[:])
                    zcur[t] = tmp

        def runs_of(ts):
            """Split chunk ids into runs contiguous in X's (b, jc) layout."""
            out = []
            for t in ts:
                if out and t == out[-1][-1] + 1 and (t % NJ) != 0:
                    out[-1].append(t)
                else:
                    out.append([t])
            return out

        def tail(ts):
            """log_cosh(w) = w + [log1p(exp(-2w)) - log2], w = z2 + x1 (in X).
            No abs needed: the identity holds for both signs and exp(-2w)
            stays <= ~e^18 here. -log2 folds into the ln via
            ln(0.5*e + 0.5) = log1p(e) - log2. w-adds run on the Pool
            engine; exp/ln batch over contiguous runs; final adds alternate
            DVE/Pool and the group's output leaves as one DMA."""
            ts = list(ts)
            k = len(ts)
            for t in ts:
                b, jc = divmod(t, NJ)
                weng = nc.vector if t % 2 == 0 else nc.gpsimd
                weng.tensor_tensor(X[:, b, jc, :], X[:, b, jc, :],
                                   zcur[t][:], op=Alu.add)
            sp = spp.tile([128, 8, D], BF, tag="sp")
            for run in runs_of(ts):
                b, jc = divmod(run[0], NJ)
                i = run[0] - ts[0]
                nc.scalar.activation(sp[:, i:i + len(run), :],
                                     X[:, b, jc:jc + len(run), :], Act.Exp,
                                     bias=zerot[:], scale=-2.0)
            nc.scalar.activation(sp[:, 0:k, :], sp[:, 0:k, :], Act.Ln,
                                 bias=halft[:], scale=0.5)
            for t in ts:
                b, jc = divmod(t, NJ)
                i = t - ts[0]
                ot = outp.tile([128, D], F32, tag="ot")
                eng = nc.gpsimd if t % 2 == 0 else nc.vector
                eng.tensor_tensor(ot[:], X[:, b, jc, :], sp[:, i, :],
                                  op=Alu.add)
                nc.sync.dma_start(out_d[b, jc * 128:(jc + 1) * 128, :], ot[:])

        # layer 1 in halves: the first half's silu lands while the PE is
        # still on the second half, so layer 2 starts without a bubble. The
        # second half's PSUM drains run on the DVE — on the in-order scalar
        # queue they would sit behind the first half's silu batch and stall
        # the PE on PSUM buffers.
        ffn_chunks(0, range(0, NT // 2), y_on_dve=False, zdt_on_scalar=True)
        ffn_epilogue(0, 0, NT // 2)
        ffn_chunks(0, range(NT // 2, NT), y_on_dve=True)
        ffn_epilogue(0, NT // 2, NT)
        # layer 2 in groups: each group's tail is emitted before the NEXT
        # group's epilogue, so it overlaps those matmuls and never waits on
        # the last silu batch.
        GROUPS = [range(0, 8), range(8, 16)]
        for gi, g in enumerate(GROUPS):
            # last group: transpose copies on the scalar engine, which idles
            # in that window while the DVE still drains earlier tails
            ffn_chunks(1, g, y_on_dve=(gi > 0),
                       zdt_on_scalar=(gi == len(GROUPS) - 1))
            if gi > 0:
                tail(GROUPS[gi - 1])
            ffn_epilogue(1, g.start, g.stop)
        tail(GROUPS[-1])

def _prep(inputs):
    x = np.asarray(inputs["x"], np.float32)
    ln1_s = np.asarray(inputs["ln1_scale"], np.float32)
    ln1_b = np.asarray(inputs["ln1_bias"], np.float32)
    Wv = np.asarray(inputs["Wv"], np.float32)
    alpha = np.asarray(inputs["alpha"], np.float32)
    Wf = np.asarray(inputs["Wf"], np.float32)
    bfv = np.asarray(inputs["bf"], np.float32)
    lnf_s = np.asarray(inputs["lnf_scale"], np.float32)
    lnf_b = np.asarray(inputs["lnf_bias"], np.float32)

    Wv_flat = Wv.transpose(1, 0, 2).reshape(D, H * HS)
    Wvp = (ln1_s[:, None] * Wv_flat).astype(BF16)
    cv = (ln1_b @ Wv_flat).astype(np.float32)

    ar = alpha[:, (-np.arange(N)) % N]
    ar2 = np.concatenate([ar, ar], axis=1)
    m_ = np.arange(NJ)[:, None, None]
    p_ = np.arange(128)[None, :, None]
    f_ = np.arange(128)[None, None, :]
    T = ar2[:, N + 128 * m_ + p_ - f_]                  # [H, NJ, 128, 128]
    tbank = np.ascontiguousarray(
        T.transpose(0, 2, 1, 3).reshape(H, 128, NJ * 128)).astype(BF16)

    cv_nonzero = bool(np.any(cv))
    bf_nonzero = tuple(bool(np.any(bfv[l])) for l in range(L))
    lnf_uniform = []
    for l in range(L):
        s, bb = lnf_s[l], lnf_b[l]
        if np.all(s == s[0]) and np.all(bb == bb[0]):
            lnf_uniform.append((float(s[0]), float(bb[0])))
        else:
            lnf_uniform.append(None)
    key = (cv_nonzero, bf_nonzero, tuple(lnf_uniform))

    common = {
        "wv": np.ascontiguousarray(Wvp),
        "wf": Wf.astype(BF16),
        "tbank": tbank,
        "id32": np.eye(128, dtype=np.float32),
        "idbf": np.eye(128, dtype=BF16),
        "cv": cv,
        "bfb": bfv,
        "lnfs": lnf_s,
        "lnfb": lnf_b,
    }
    return x, key, common, (cv_nonzero, bf_nonzero, lnf_uniform)


def kernel(**inputs):
    x, key, common, flags = _prep(inputs)
    if key not in _cache:
        _cache[key] = _build(*flags)
    nc = _cache[key]
    in_maps = []
    for i in range(NCORES):
        m = dict(common)
        m["xs"] = np.ascontiguousarray(x[i * BPC:(i + 1) * BPC])
        in_maps.append(m)
    res = run_bass_kernel_spmd(nc, in_maps, core_ids=list(range(NCORES)),
                               trace=TRACE, **TRACE_KW)
    kernel.last_result = res
    out = np.empty((B, N, D), np.float32)
    for i in range(NCORES):
        out[i * BPC:(i + 1) * BPC] = res.results[i]["out"]
    return out


# revision 30
# speedup vs baseline: 1.0852x; 1.0009x over previous
"""Trainium2 Bass kernel for nn_CoreBlock (circulant attention + 2-layer FFN).

Contract: kernel(**inputs) takes FULL unsharded inputs (as produced by
setup_inputs) and returns the FULL [16, 1024, 768] f32 output.

Strategy: pure data-parallel over batch — 8 NeuronCores x 2 batches each.
All weights replicated. Per core:
  phase A: LayerNorm(x) -> u (stats on DVE, rstd via a DVE Newton rsqrt,
           normalize on the ACT engine with per-token scale/bias),
           PE-transpose u, v = u_dt.T @ Wv' per token chunk, gathered into
           a resident V tensor [128, H, B, NJ, HS] in SBUF. The Toeplitz
           bank / FFN weight DMAs are queued behind batch 0's x chunks so
           the ramp-critical loads get the HBM bandwidth first.
  phase B: per head h and batch b: circulant matmul y[ic] = sum_m T[m] @
           v[(ic+m)%8]; each diagonal m is 2 wide matmuls (moving free up
           to 512) so the PE streams instead of reloading weights per
           64-wide product. Residual-added into X (X becomes x1).
  phase C: 2x [Dense -> LayerNorm -> swish], token-major with PE
           transposes between layers. Layer 1 runs in halves, layer 2 in
           groups of [7, 6, 3]: each group's log_cosh tail overlaps the
           next group's matmuls. Groups after the first route their PSUM
           drains to the DVE so the in-order scalar queue can keep draining
           the previous tail. Tail: log_cosh(w) = w + [log1p(exp(-2w)) -
           log2] with -log2 folded into the ln (ln(0.5e + 0.5)); exp ops
           and ln ops batch separately (the runtime assigns them different
           activation tables); w-adds and final adds alternate DVE/Pool.

Matmul operands are bf16 (full-rate PE, fp32 PSUM accumulation); stats and
elementwise math are fp32. The scalar engine's activation tables only ever
cycle between Silu and Exp/Ln (sqrt lives on the DVE), keeping table
reloads off the critical path.
"""

import math
import numpy as np
import ml_dtypes

import concourse.bass as bass
import concourse.tile as tile
from concourse import bacc, mybir
from concourse.bass_utils import run_bass_kernel_spmd

BF16 = ml_dtypes.bfloat16

B, N, D = 16, 1024, 768
H, HS, L = 12, 64, 2
EPS = 1e-6
NCORES = 8
BPC = B // NCORES          # batches per core
NJ = N // 128              # token chunks per batch (8)
NT = BPC * NJ              # token chunks per core (16)
DC = D // 128              # feature chunks (6)

F32 = mybir.dt.float32
BF = mybir.dt.bfloat16
Alu = mybir.AluOpType
Act = mybir.ActivationFunctionType

TRACE = False              # test harness sets this for profiling runs
TRACE_KW = {}

_cache = {}


def _build(cv_nonzero, bf_nonzero, lnf_uniform):
    """Construct the per-core Bass program. lnf_uniform: per-layer (cs, cb)
    if lnf scale/bias are uniform scalars, else None entries."""
    nc = bacc.Bacc("TRN2", target_bir_lowering=False, debug=False)

    xs = nc.dram_tensor("xs", (BPC, N, D), F32, kind="ExternalInput").ap()
    wv = nc.dram_tensor("wv", (D, D), BF, kind="ExternalInput").ap()
    wf = nc.dram_tensor("wf", (L, D, D), BF, kind="ExternalInput").ap()
    tb_d = nc.dram_tensor("tbank", (H, 128, NJ * 128), BF, kind="ExternalInput").ap()
    id32 = nc.dram_tensor("id32", (128, 128), F32, kind="ExternalInput").ap()
    idbf = nc.dram_tensor("idbf", (128, 128), BF, kind="ExternalInput").ap()
    cv_d = nc.dram_tensor("cv", (D,), F32, kind="ExternalInput").ap()
    bf_d = nc.dram_tensor("bfb", (L, D), F32, kind="ExternalInput").ap()
    lnfs_d = nc.dram_tensor("lnfs", (L, D), F32, kind="ExternalInput").ap()
    lnfb_d = nc.dram_tensor("lnfb", (L, D), F32, kind="ExternalInput").ap()
    out_d = nc.dram_tensor("out", (BPC, N, D), F32, kind="ExternalOutput").ap()

    with tile.TileContext(nc) as tc:
        _emit(nc, tc, xs, wv, wf, tb_d, id32, idbf, cv_d, bf_d, lnfs_d, lnfb_d,
              out_d, cv_nonzero, bf_nonzero, lnf_uniform)
    nc.compile()
    return nc


def _emit(nc, tc, xs, wv, wf, tb_d, id32, idbf, cv_d, bf_d, lnfs_d, lnfb_d,
          out_d, cv_nonzero, bf_nonzero, lnf_uniform):
    from contextlib import ExitStack
    ctx = ExitStack()
    with ctx:
        consts = ctx.enter_context(tc.tile_pool(name="consts", bufs=1))
        xpool = ctx.enter_context(tc.tile_pool(name="xpool", bufs=1))
        vpool = ctx.enter_context(tc.tile_pool(name="vpool", bufs=1))
        acts = ctx.enter_context(tc.tile_pool(name="acts", bufs=18))
        upool = ctx.enter_context(tc.tile_pool(name="upool", bufs=3))
        dtp = ctx.enter_context(tc.tile_pool(name="dtp", bufs=3))
        stat = ctx.enter_context(tc.tile_pool(name="stat", bufs=4))
        statp = ctx.enter_context(tc.tile_pool(name="statp", bufs=2))
        wkp = ctx.enter_context(tc.tile_pool(name="wkp", bufs=3))
        spp = ctx.enter_context(tc.tile_pool(name="spp", bufs=2))
        outp = ctx.enter_context(tc.tile_pool(name="outp", bufs=3))
        ps_tr = ctx.enter_context(tc.tile_pool(name="ps_tr", bufs=2, space="PSUM"))
        ps_mm = ctx.enter_context(tc.tile_pool(name="ps_mm", bufs=2, space="PSUM"))

        # ---- constants (weights the PE needs first are DMA'd first) ----
        wv_s = consts.tile([128, DC, D], BF, tag="wv")
        wf_s = consts.tile([128, L, DC, D], BF, tag="wf")
        tb_s = consts.tile([128, H, NJ, 128], BF, tag="tb")
        i32 = consts.tile([128, 128], F32, tag="i32")
        ibf = consts.tile([128, 128], BF, tag="ibf")
        nc.sync.dma_start(ibf[:], idbf)
        nc.sync.dma_start(i32[:], id32)
        nc.sync.dma_start(wv_s[:], wv.rearrange("(c p) f -> p c f", p=128))
        epst = consts.tile([128, 1], F32, tag="eps")
        nc.vector.memset(epst[:], EPS)
        zerot = consts.tile([128, 1], F32, tag="zero")
        nc.vector.memset(zerot[:], 0.0)
        onet = consts.tile([128, 1], F32, tag="one")
        nc.vector.memset(onet[:], 1.0)
        halft = consts.tile([128, 1], F32, tag="half")
        nc.vector.memset(halft[:], 0.5)
        cvt = None
        if cv_nonzero:
            cvt = consts.tile([128, D], F32, tag="cv")
            nc.sync.dma_start(cvt[:], cv_d.to_broadcast((128, D)))
        bft = [None] * L
        lnfst = [None] * L
        lnfbt = [None] * L
        for l in range(L):
            if bf_nonzero[l]:
                bft[l] = consts.tile([128, D], F32, tag=f"bf{l}")
                nc.sync.dma_start(bft[l][:], bf_d[l].to_broadcast((128, D)))
            if lnf_uniform[l] is None:
                lnfst[l] = consts.tile([128, D], F32, tag=f"lnfs{l}")
                nc.sync.dma_start(lnfst[l][:], lnfs_d[l].to_broadcast((128, D)))
                lnfbt[l] = consts.tile([128, D], F32, tag=f"lnfb{l}")
                nc.sync.dma_start(lnfbt[l][:], lnfb_d[l].to_broadcast((128, D)))

        # ---- resident tensors ----
        X = xpool.tile([128, BPC, NJ, D], F32, tag="X")         # x, then x1
        V = vpool.tile([128, H, BPC, NJ, HS], BF, tag="V")      # per-head values

        I32i = mybir.dt.int32

        def emit_rsqrt(dst, var, uniq):
            """rstd = 1/sqrt(var + EPS) entirely on the DVE: fast-inverse-
            sqrt bit trick + 2 Newton iterations (rel err ~5e-6). Keeps the
            scalar engine's activation table free for Silu/Exp/Ln."""
            n = dst.shape[-1]
            ve = stat.tile([128, NT], F32, tag="rq_ve", name="rq_ve")[:, 0:n]
            hv = stat.tile([128, NT], F32, tag="rq_hv", name="rq_hv")[:, 0:n]
            ri = stat.tile([128, NT], I32i, tag="rq_ri", name="rq_ri")[:, 0:n]
            nc.vector.tensor_scalar(ve, var, EPS, None, op0=Alu.add)
            nc.vector.tensor_scalar(hv, ve, 0.5, None, op0=Alu.mult)
            nc.vector.tensor_scalar(ri, ve.bitcast(I32i), 1, None,
                                    op0=Alu.arith_shift_right)
            nc.vector.tensor_scalar(ri, ri, -1, 0x5f3759df,
                                    op0=Alu.mult, op1=Alu.add)
            cur = ri.bitcast(F32)
            for it in range(2):
                t1 = stat.tile([128, NT], F32, tag=f"rq_t{it}",
                               name=f"rq_t{it}")[:, 0:n]
                nc.vector.tensor_tensor(t1, cur, cur, op=Alu.mult)
                nc.vector.tensor_tensor(t1, t1, hv, op=Alu.mult)
                nc.vector.tensor_scalar(t1, t1, -1.0, 1.5,
                                        op0=Alu.mult, op1=Alu.add)
                out = dst if it == 1 else stat.tile(
                    [128, NT], F32, tag="rq_c", name="rq_c")[:, 0:n]
                nc.vector.tensor_tensor(out, cur, t1, op=Alu.mult)
                cur = out

        # ================= phase A: LN + v-projection =================
        # Per batch: 8 chunks in sub-batches of 4 (stats -> packed DVE rsqrt
        # -> normalize on ACT + transpose + project), then phase B (circulant
        # + residual) for that batch so the pipeline stays at half depth.
        mvA = statp.tile([128, NT, 2], F32, tag="mvA")
        rsA = statp.tile([128, NT], F32, tag="rsA")
        nbA = statp.tile([128, NT], F32, tag="nbA")
        AB = 4
        for t0 in range(0, NT, AB):
            for t in range(t0, t0 + AB):
                b, jc = divmod(t, NJ)
                xt = X[:, b, jc, :]
                nc.sync.dma_start(xt, xs[b, jc * 128:(jc + 1) * 128, :])
                st = stat.tile([128, 3, 6], F32, tag="bst")
                for g in range(3):
                    nc.vector.bn_stats(st[:, g, :], xt[:, g * 256:(g + 1) * 256])
                nc.vector.bn_aggr(mvA[:, t, :], st[:])
            emit_rsqrt(rsA[:, t0:t0 + AB], mvA[:, t0:t0 + AB, 1], t0)
            # per-token -mu*rstd for the ACT-engine normalize, one batched op
            nc.vector.scalar_tensor_tensor(
                nbA[:, t0:t0 + AB], mvA[:, t0:t0 + AB, 0], -1.0,
                rsA[:, t0:t0 + AB], op0=Alu.mult, op1=Alu.mult)
            from contextlib import nullcontext
            prio = tc.high_priority() if t0 == 0 else nullcontext()
            with prio:
                for t in range(t0, t0 + AB):
                    b, jc = divmod(t, NJ)
                    xt = X[:, b, jc, :]
                    u = upool.tile([128, D], BF, tag="u")
                    nc.scalar.activation(u[:], xt, Act.Identity,
                                         bias=nbA[:, t:t + 1],
                                         scale=rsA[:, t:t + 1])
                    ptr = ps_tr.tile([128, D], BF, tag="tr")
                    for c in range(DC):
                        nc.tensor.transpose(ptr[:, c * 128:(c + 1) * 128],
                                            u[:, c * 128:(c + 1) * 128], ibf[:])
                    udt = dtp.tile([128, D], BF, tag="udt")
                    nc.scalar.copy(udt[:], ptr[:])
                    pv = ps_mm.tile([128, D], F32, tag="mm")
                    for c in range(DC):
                        nc.tensor.matmul(pv[:, 0:512],
                                         udt[:, c * 128:(c + 1) * 128],
                                         wv_s[:, c, 0:512],
                                         start=(c == 0), stop=(c == DC - 1))
                        nc.tensor.matmul(pv[:, 512:D],
                                         udt[:, c * 128:(c + 1) * 128],
                                         wv_s[:, c, 512:D],
                                         start=(c == 0), stop=(c == DC - 1))
                    vdst = V[:, :, b, jc, :]                         # [128, H, HS]
                    pv3 = pv[:].rearrange("p (h k) -> p h k", h=H)
                    if cv_nonzero:
                        cv3 = cvt[:].rearrange("p (h k) -> p h k", h=H)
                        nc.vector.tensor_tensor(vdst, pv3, cv3, op=Alu.add)
                    elif t % 2 == 0:
                        # alternate the PSUM->SBUF copy between DVE and ACT
                        nc.vector.tensor_copy(vdst, pv3)
                    else:
                        nc.scalar.copy(vdst, pv3)
            if t0 // AB == 1:
                # Toeplitz bank on the SP queue AFTER batch 0's x-chunk
                # issues: the SP issues in order, so the bank transfer can't
                # steal HBM bandwidth from the stats-critical first loads.
                for h in range(H):
                    nc.sync.dma_start(
                        tb_s[:, h, :, :],
                        tb_d[h].rearrange("p (m f) -> p m f", m=NJ))
            if t0 // AB == 2:
                nc.sync.dma_start(
                    wf_s[:, 0, :, :], wf[0].rearrange("(c p) f -> p c f", p=128))
            if t0 // AB == 3:
                nc.sync.dma_start(
                    wf_s[:, 1, :, :], wf[1].rearrange("(c p) f -> p c f", p=128))
            # ======== phase B for batch bb once its 8 chunks are in V ======
            # y[ic] = sum_m T[m] @ v[(ic+m)%8]: for each diagonal m the ic
            # range splits into two contiguous runs, so the moving operand is
            # up to 512 wide and the PE streams instead of reloading weights.
            if (t0 + AB) % NJ == 0:
                bb = (t0 + AB) // NJ - 1
                for h in range(H):
                    pc = ps_mm.tile([128, NJ, HS], F32, tag="mm")
                    for m in range(NJ):
                        k = NJ - m
                        nc.tensor.matmul(pc[:, 0:k, :], tb_s[:, h, m, :],
                                         V[:, h, bb, m:NJ, :],
                                         start=(m == 0), stop=(m == NJ - 1),
                                         skip_group_check=True)
                        if m > 0:
                            nc.tensor.matmul(pc[:, k:NJ, :], tb_s[:, h, m, :],
                                             V[:, h, bb, 0:m, :],
                                             start=False, stop=(m == NJ - 1),
                                             skip_group_check=True)
                    xap = X[:, bb, :, h * HS:(h + 1) * HS]           # [128,NJ,HS]
                    nc.vector.tensor_tensor(xap, xap, pc[:], op=Alu.add)

        # ================= phase C: FFN x2 + log_cosh =================
        # Token-major FFN: per chunk, PE-transpose the input, Dense matmul
        # into PSUM, copy+row-sum to SBUF, sumsq on the bf16 copy, batched
        # DVE rsqrt, then Silu with per-token scale/bias. Layer 2 runs in
        # groups so each group's log_cosh tail overlaps the next group's
        # matmuls; exp/ln are batched per group (they live in one activation
        # table, but separate tables from Silu).
        inv_d = 1.0 / D
        zcur = [None] * NT
        sums = [statp.tile([128, NT], F32, tag=f"sum{l}", name=f"sum{l}")
                for l in range(L)]
        ssq = [statp.tile([128, NT], F32, tag=f"ssq{l}", name=f"ssq{l}")
               for l in range(L)]
        rsF = [statp.tile([128, NT], F32, tag=f"rs{l}", name=f"rs{l}")
               for l in range(L)]
        biasF = [statp.tile([128, NT], F32, tag=f"bi{l}", name=f"bi{l}")
                 for l in range(L)]

        def ffn_chunks(l, ts, y_on_dve, zdt_on_scalar=False):
            """Dense matmuls + sum/sumsq accumulation for chunks ts of
            layer l. y_on_dve routes the PSUM->SBUF copy to the DVE so the
            scalar engine stays free for the previous group's tail."""
            for t in ts:
                b, jc = divmod(t, NJ)
                src_ = X[:, b, jc, :] if l == 0 else zcur[t][:]
                ptr = ps_tr.tile([128, D], F32 if l == 0 else BF, tag="tr")
                ident = i32 if l == 0 else ibf
                for c in range(DC):
                    nc.tensor.transpose(ptr[:, c * 128:(c + 1) * 128],
                                        src_[:, c * 128:(c + 1) * 128], ident[:])
                zdt = dtp.tile([128, D], BF, tag="zdt")
                if zdt_on_scalar:
                    nc.scalar.copy(zdt[:], ptr[:])
                else:
                    nc.vector.tensor_copy(zdt[:], ptr[:])
                pf = ps_mm.tile([128, D], F32, tag="mm")
                for c in range(DC):
                    nc.tensor.matmul(pf[:, 0:512], zdt[:, c * 128:(c + 1) * 128],
                                     wf_s[:, l, c, 0:512],
                                     start=(c == 0), stop=(c == DC - 1))
                    nc.tensor.matmul(pf[:, 512:D], zdt[:, c * 128:(c + 1) * 128],
                                     wf_s[:, l, c, 512:D],
                                     start=(c == 0), stop=(c == DC - 1))
                if bf_nonzero[l]:
                    nc.vector.tensor_tensor(pf[:], pf[:], bft[l][:], op=Alu.add)
                y = acts.tile([128, D], BF, tag="acts")
                if y_on_dve:
                    nc.vector.tensor_scalar(y[:], pf[:], 1.0, 0.0, op0=Alu.mult,
                                            op1=Alu.add,
                                            accum_out=sums[l][:, t:t + 1])
                else:
                    nc.scalar.activation(y[:], pf[:], Act.Copy,
                                         accum_out=sums[l][:, t:t + 1])
                scr = wkp.tile([128, D], BF, tag="scr")
                nc.vector.scalar_tensor_tensor(
                    scr[:], y[:], 0.0, y[:], op0=Alu.add, op1=Alu.mult,
                    accum_out=ssq[l][:, t:t + 1])
                zcur[t] = y

        def ffn_epilogue(l, t0, t1):
            """Batched LN stats -> DVE rstd/bias -> silu for chunks [t0, t1)."""
            sl = slice(t0, t1)
            mu = stat.tile([128, NT], F32, tag="muE")
            var = stat.tile([128, NT], F32, tag="varE")
            nc.vector.tensor_scalar(mu[:, sl], sums[l][:, sl], inv_d, None,
                                    op0=Alu.mult)
            nc.vector.scalar_tensor_tensor(var[:, sl], mu[:, sl], -1.0,
                                           mu[:, sl], op0=Alu.mult, op1=Alu.mult)
            nc.vector.scalar_tensor_tensor(var[:, sl], ssq[l][:, sl], inv_d,
                                           var[:, sl], op0=Alu.mult, op1=Alu.add)
            emit_rsqrt(rsF[l][:, sl], var[:, sl], 100 + l * 16 + t0)
            fast = lnf_uniform[l] is not None
            if fast:
                cs, cb = lnf_uniform[l]
                if cs != 1.0:
                    nc.vector.tensor_scalar(rsF[l][:, sl], rsF[l][:, sl],
                                            float(cs), None, op0=Alu.mult)
                nc.vector.scalar_tensor_tensor(biasF[l][:, sl], mu[:, sl], -1.0,
                                               rsF[l][:, sl],
                                               op0=Alu.mult, op1=Alu.mult)
                if cb != 0.0:
                    nc.vector.tensor_scalar(biasF[l][:, sl], biasF[l][:, sl],
                                            float(cb), None, op0=Alu.add)
                for t in range(t0, t1):
                    y = zcur[t]
                    nc.scalar.activation(y[:], y[:], Act.Silu,
                                         bias=biasF[l][:, t:t + 1],
                                         scale=rsF[l][:, t:t + 1])
            else:
                for t in range(t0, t1):
                    y = zcur[t]
                    tmp = acts.tile([128, D], BF, tag="acts")
                    nc.vector.tensor_scalar(tmp[:], y[:], mu[:, t:t + 1],
                                            rsF[l][:, t:t + 1],
                                            op0=Alu.subtract, op1=Alu.mult)
                    nc.vector.tensor_tensor(tmp[:], tmp[:], lnfst[l][:],
                                            op=Alu.mult)
                    nc.vector.tensor_tensor(tmp[:], tmp[:], lnfbt[l][:],
                                            op=Alu.add)
                    nc.scalar.activation(tmp[:], tmp[:], Act.Silu,
                                         bias=zerot[:])
                    zcur[t] = tmp

        def runs_of(ts):
            """Split chunk ids into runs contiguous in X's (b, jc) layout."""
            out = []
            for t in ts:
                if out and t == out[-1][-1] + 1 and (t % NJ) != 0:
                    out[-1].append(t)
                else:
                    out.append([t])
            return out

        def tail(ts):
            """log_cosh(w) = w + [log1p(exp(-2w)) - log2], w = z2 + x1 (in X).
            No abs needed: the identity holds for both signs and exp(-2w)
            stays <= ~e^18 here. -log2 folds into the ln via
            ln(0.5*e + 0.5) = log1p(e) - log2. w-adds run on the Pool
            engine; exp/ln batch over contiguous runs; final adds alternate
            DVE/Pool and the group's output leaves as one DMA."""
            ts = list(ts)
            k = len(ts)
            for t in ts:
                b, jc = divmod(t, NJ)
                weng = nc.vector if t % 2 == 0 else nc.gpsimd
                weng.tensor_tensor(X[:, b, jc, :], X[:, b, jc, :],
                                   zcur[t][:], op=Alu.add)
            sp = spp.tile([128, 8, D], BF, tag="sp")
            for run in runs_of(ts):
                b, jc = divmod(run[0], NJ)
                i = run[0] - ts[0]
                nc.scalar.activation(sp[:, i:i + len(run), :],
                                     X[:, b, jc:jc + len(run), :], Act.Exp,
                                     bias=zerot[:], scale=-2.0)
            nc.scalar.activation(sp[:, 0:k, :], sp[:, 0:k, :], Act.Ln,
                                 bias=halft[:], scale=0.5)
            for t in ts:
                b, jc = divmod(t, NJ)
                i = t - ts[0]
                ot = outp.tile([128, D], F32, tag="ot")
                eng = nc.gpsimd if t % 2 == 0 else nc.vector
                eng.tensor_tensor(ot[:], X[:, b, jc, :], sp[:, i, :],
                                  op=Alu.add)
                nc.sync.dma_start(out_d[b, jc * 128:(jc + 1) * 128, :], ot[:])

        # layer 1 in halves: the first half's silu lands while the PE is
        # still on the second half, so layer 2 starts without a bubble. The
        # second half's PSUM drains run on the DVE — on the in-order scalar
        # queue they would sit behind the first half's silu batch and stall
        # the PE on PSUM buffers.
        ffn_chunks(0, range(0, NT // 2), y_on_dve=False, zdt_on_scalar=True)
        ffn_epilogue(0, 0, NT // 2)
        ffn_chunks(0, range(NT // 2, NT), y_on_dve=True)
        ffn_epilogue(0, NT // 2, NT)
        # layer 2 in groups: each group's tail is emitted before the NEXT
        # group's epilogue, so it overlaps those matmuls and never waits on
        # the last silu batch.
        GROUPS = [range(0, 8), range(8, 16)]
        for gi, g in enumerate(GROUPS):
            ffn_chunks(1, g, y_on_dve=(gi > 0))
            if gi > 0:
                tail(GROUPS[gi - 1])
            ffn_epilogue(1, g.start, g.stop)
        tail(GROUPS[-1])

def _prep(inputs):
    x = np.asarray(inputs["x"], np.float32)
    ln1_s = np.asarray(inputs["ln1_scale"], np.float32)
    ln1_b = np.asarray(inputs["ln1_bias"], np.float32)
    Wv = np.asarray(inputs["Wv"], np.float32)
    alpha = np.asarray(inputs["alpha"], np.float32)
    Wf = np.asarray(inputs["Wf"], np.float32)
    bfv = np.asarray(inputs["bf"], np.float32)
    lnf_s = np.asarray(inputs["lnf_scale"], np.float32)
    lnf_b = np.asarray(inputs["lnf_bias"], np.float32)

    Wv_flat = Wv.transpose(1, 0, 2).reshape(D, H * HS)
    Wvp = (ln1_s[:, None] * Wv_flat).astype(BF16)
    cv = (ln1_b @ Wv_flat).astype(np.float32)

    ar = alpha[:, (-np.arange(N)) % N]
    ar2 = np.concatenate([ar, ar], axis=1)
    m_ = np.arange(NJ)[:, None, None]
    p_ = np.arange(128)[None, :, None]
    f_ = np.arange(128)[None, None, :]
    T = ar2[:, N + 128 * m_ + p_ - f_]                  # [H, NJ, 128, 128]
    tbank = np.ascontiguousarray(
        T.transpose(0, 2, 1, 3).reshape(H, 128, NJ * 128)).astype(BF16)

    cv_nonzero = bool(np.any(cv))
    bf_nonzero = tuple(bool(np.any(bfv[l])) for l in range(L))
    lnf_uniform = []
    for l in range(L):
        s, bb = lnf_s[l], lnf_b[l]
        if np.all(s == s[0]) and np.all(bb == bb[0]):
            lnf_uniform.append((float(s[0]), float(bb[0])))
        else:
            lnf_uniform.append(None)
    key = (cv_nonzero, bf_nonzero, tuple(lnf_uniform))

    common = {
        "wv": np.ascontiguousarray(Wvp),
        "wf": Wf.astype(BF16),
        "tbank": tbank,
        "id32": np.eye(128, dtype=np.float32),
        "idbf": np.eye(128, dtype=BF16),
        "cv": cv,
        "bfb": bfv,
        "lnfs": lnf_s,
        "lnfb": lnf_b,
    }
    return x, key, common, (cv_nonzero, bf_nonzero, lnf_uniform)


def kernel(**inputs):
    x, key, common, flags = _prep(inputs)
    if key not in _cache:
        _cache[key] = _build(*flags)
    nc = _cache[key]
    in_maps = []
    for i in range(NCORES):
        m = dict(common)
        m["xs"] = np.ascontiguousarray(x[i * BPC:(i + 1) * BPC])
        in_maps.append(m)
    res = run_bass_kernel_spmd(nc, in_maps, core_ids=list(range(NCORES)),
                               trace=TRACE, **TRACE_KW)
    kernel.last_result = res
    out = np.empty((B, N, D), np.float32)
    for i in range(NCORES):
        out[i * BPC:(i + 1) * BPC] = res.results[i]["out"]
    return out
